# revision 14
# baseline (speedup 1.0000x reference)
"""BEVFormer spatial cross-attention encoder kernel for Trainium2 (8 NeuronCores).

Contract: kernel(**inputs) takes FULL unsharded inputs (feat, I, E, grid_3d),
shards queries across 8 cores, runs a Bass/Tile kernel per core, and returns
the FULL (1, 22500, 128) output.

Optimizations vs the naive 24-slot/2-descriptor version:
  * A BEV query can be inside the (u, z)-frustum of at most 2 of the 6
    outward-facing cameras, and that validity depends only on (x, y) — not
    depth. The host selects the <=2 candidate cameras per query and builds
    per-query projection coefficients, so the device only projects/gathers
    8 (cam, depth) slots instead of 24.
  * Gather descriptors cost ~40ns each on the (chip-shared) DMA engines
    regardless of size, so the host pre-packs each pixel's 2x2 bilinear
    footprint contiguously (feat_quad[pix, 4, C], bf16) and the device
    fetches one 1KB descriptor per (query, slot) instead of two row pairs.
    Quad-edge clamping is folded into the bilinear weights with the same
    shift trick used for the x-pair.

Per-core device program:
  1. Project 8 slots x query points with per-query selected affine coeffs
     (DVE), build masks, bilinear tap weights and gather indices on-device.
  2. PE-transpose weights to query-on-partition layout.
  3. dma_gather 2x2 quads from HBM (1 descriptor per point).
  4. Fused multiply-accumulate (scalar_tensor_tensor) into the output tile.
  5. Normalize by the mask count, DMA out.
"""
import os
import numpy as np

# ---- problem constants (hardcoded per contract) ----
NCAM = 6
DD = 4
NSLOT = 2 * DD          # 8 (cam-rank, depth) slots
NLIST = NSLOT           # 8 gather lists (one quad per slot)
FH = 48
FW = 88
C = 128
PIX = FH * FW           # 4224
NPIX = NCAM * PIX       # 25344
BEV_H = 150
BEV_W = 150
QTOT = BEV_H * BEV_W    # 22500
NCORES = 8
QCORE = 2816            # 22 * 128
QPAD = NCORES * QCORE   # 22528
NCHUNK = QCORE // 128   # 22
IMG_W = 800.0
IMG_H = 480.0
PC = np.array([-51.2, -51.2, -5.0, 51.2, 51.2, 3.0], np.float64)
EPS = 1e-5
MAGIC = 12582912.0      # 3 * 2^22: f32 round-to-int magic

HALF = QCORE // 2       # pipeline processed in 2 halves of 1408 queries
NCH = HALF // 128       # 11 chunks per half
NW = 5 * NSLOT          # 40 payload columns per chunk in wT

_CACHE = {}


def _build_program():
    import concourse.bacc as bacc
    import concourse.bass as bass
    import concourse.mybir as mybir
    import concourse.tile as tile
    from concourse import masks
    from concourse import library_config
    from concourse.alu_op_type import AluOpType as op

    f32 = mybir.dt.float32
    i16 = mybir.dt.int16
    use_f32 = bool(os.environ.get("BASS_KERNEL_F32"))
    fdt = f32 if use_f32 else mybir.dt.bfloat16

    nc = bacc.Bacc("TRN2", target_bir_lowering=False, debug=False, num_swdge_queues=4)

    feat = nc.dram_tensor("feat", [NPIX, 4 * C], fdt, kind="ExternalInput")
    grid = nc.dram_tensor("grid", [3, DD, QCORE], f32, kind="ExternalInput")
    # per-query selected coefficients: [13 coef, 2 cam-rank, QCORE]
    coef = nc.dram_tensor("coef", [13, 2, QCORE], f32, kind="ExternalInput")
    outd = nc.dram_tensor("out", [QCORE, C], f32, kind="ExternalOutput")

    featAP = bass.AP(feat, 0, [[4 * C, NPIX], [1, 4 * C]])

    with tile.TileContext(nc) as tc:
        with tc.tile_pool(name="persist", bufs=1) as pp, \
             tc.tile_pool(name="dram", bufs=1, space="DRAM") as dp, \
             tc.tile_pool(name="psum", bufs=2, space="PSUM") as psp:

            nc.gpsimd.load_library(library_config.mlp)

            ident = pp.tile([128, 128], f32)
            masks.make_identity(nc, ident[:])

            # weights transposed to q-on-partition: [128, 22 chunks, 5*8]
            wT = pp.tile([128, NCHUNK * NW], f32)
            # wrapped gather index lists: [16-part groups, half, 8 lists * q/16]
            idxw = pp.tile([128, 2, NLIST * 88], i16)
            accA = pp.tile([128, NCHUNK, C], f32)
            accB = pp.tile([128, NCHUNK, C], f32)
            nc.vector.memset(accA[:], 0.0)
            acc_cur, acc_nxt = accA, accB
            cnt = pp.tile([128, NCHUNK], f32)
            rec = pp.tile([128, NCHUNK], f32)

            idx_dram = dp.tile([2, NLIST, HALF], i16)
            gA_tiles = []

            # ---------------- stage 1: projection pipeline (2 halves) -------
            with tc.tile_pool(name="pipe", bufs=1) as pl:
                for h in range(2):
                    q0 = h * HALF
                    S = lambda k: pl.tile([NSLOT, HALF], f32, tag=f"s{k}", name=f"s{k}")

                    # per-query selected coefficients, slot p=r*4+d reads [k, r, q]
                    cf = pl.tile([NSLOT, 13, HALF], f32, tag="cf", name="cf")
                    for k in range(13):
                        src = bass.AP(coef, k * 2 * QCORE + q0,
                                      [[QCORE, 2], [0, DD], [1, HALF]])
                        nc.scalar.dma_start(cf[:, k, :], src)
                    ck = lambda k: cf[:, k, :]

                    Xb = S(0); Yb = S(1); Zb = S(2)
                    # broadcast rows: partition p = r*4+d reads grid[comp, d, q]
                    for t, comp in ((Xb, 0), (Yb, 1), (Zb, 2)):
                        src = bass.AP(grid, comp * DD * QCORE + q0,
                                      [[0, 2], [QCORE, DD], [1, HALF]])
                        nc.scalar.dma_start(t[:], src)

                    t0 = S(6)
                    XC = S(3); YC = S(4); ZC = S(5)
                    for dst, cb in ((XC, 0), (YC, 4), (ZC, 8)):
                        nc.vector.tensor_tensor(dst[:], Xb[:], ck(cb + 0), op.mult)
                        nc.vector.tensor_tensor(t0[:], Yb[:], ck(cb + 1), op.mult)
                        nc.vector.tensor_tensor(dst[:], dst[:], t0[:], op.add)
                        nc.vector.tensor_tensor(t0[:], Zb[:], ck(cb + 2), op.mult)
                        nc.vector.tensor_tensor(dst[:], dst[:], t0[:], op.add)
                        nc.vector.tensor_tensor(dst[:], dst[:], ck(cb + 3), op.add)

                    # mask (slots 0,1 recycled as scratch, 2 as M)
                    t0 = S(0); t1 = S(1); M = S(2)
                    nc.vector.tensor_scalar_mul(t0[:], ZC[:], IMG_W)
                    nc.vector.tensor_tensor(t1[:], XC[:], t0[:], op.is_lt)
                    nc.vector.scalar_tensor_tensor(M[:], XC[:], 0.0, t1[:], op.is_gt, op.mult)
                    nc.vector.tensor_scalar_mul(t0[:], ZC[:], IMG_H)
                    nc.vector.tensor_tensor(t1[:], YC[:], t0[:], op.is_lt)
                    nc.vector.tensor_tensor(t1[:], t1[:], M[:], op.mult)
                    nc.vector.scalar_tensor_tensor(M[:], YC[:], 0.0, t1[:], op.is_gt, op.mult)
                    nc.vector.scalar_tensor_tensor(M[:], ZC[:], EPS, M[:], op.is_gt, op.mult)

                    # px, py
                    invz = S(6)
                    nc.vector.tensor_scalar_max(t0[:], ZC[:], EPS)
                    nc.vector.reciprocal(invz[:], t0[:])
                    px = S(5)  # reuse ZC slot
                    nc.vector.tensor_tensor(t0[:], XC[:], invz[:], op.mult)
                    nc.vector.tensor_scalar(px[:], t0[:], FW / IMG_W, -0.5, op.mult, op.add)
                    py = S(7)
                    nc.vector.tensor_tensor(t0[:], YC[:], invz[:], op.mult)
                    nc.vector.tensor_scalar(py[:], t0[:], FH / IMG_H, -0.5, op.mult, op.add)

                    # floor(x) = RNE((x - 0.5) + MAGIC) - MAGIC (ties harmless)
                    x0f = S(3); y0f = S(4)  # reuse XC/YC slots
                    nc.vector.tensor_single_scalar(t1[:], px[:], 0.5, op.subtract)
                    nc.vector.tensor_scalar(x0f[:], t1[:], MAGIC, MAGIC, op.add, op.subtract)
                    nc.vector.tensor_single_scalar(t1[:], py[:], 0.5, op.subtract)
                    nc.vector.tensor_scalar(y0f[:], t1[:], MAGIC, MAGIC, op.add, op.subtract)

                    wx1 = S(6); wy1 = S(8)  # invz dead
                    nc.vector.tensor_tensor(wx1[:], px[:], x0f[:], op.subtract)
                    nc.vector.tensor_tensor(wy1[:], py[:], y0f[:], op.subtract)

                    # x pair: clamp start, shift-aware tap weights
                    xs = S(7); o = S(5)  # py, px dead
                    nc.vector.tensor_scalar(xs[:], x0f[:], 0.0, float(FW - 2), op.max, op.min)
                    nc.vector.tensor_tensor(o[:], x0f[:], xs[:], op.subtract)
                    wx0 = S(9)
                    nc.vector.tensor_scalar(wx0[:], wx1[:], -1.0, 1.0, op.mult, op.add)
                    # wL = wx0*(o==0) + wx1*(o==-1) ; wR = wx1*(o==0) + wx0*(o==1)
                    e0 = S(10)
                    nc.vector.tensor_single_scalar(e0[:], o[:], 0.0, op.is_equal)
                    wL = S(11); wR = S(12)
                    nc.vector.tensor_tensor(wL[:], wx0[:], e0[:], op.mult)
                    nc.vector.tensor_tensor(wR[:], wx1[:], e0[:], op.mult)
                    nc.vector.tensor_single_scalar(e0[:], o[:], -1.0, op.is_equal)
                    nc.vector.tensor_tensor(t0[:], wx1[:], e0[:], op.mult)
                    nc.vector.tensor_tensor(wL[:], wL[:], t0[:], op.add)
                    nc.vector.tensor_single_scalar(e0[:], o[:], 1.0, op.is_equal)
                    nc.vector.tensor_tensor(t0[:], wx0[:], e0[:], op.mult)
                    nc.vector.tensor_tensor(wR[:], wR[:], t0[:], op.add)

                    # y quad base: clamp to [0, FH-2], shift-aware row weights
                    # wTop = (1-wy1)*M*(oy==0) + wy1*M*(oy==-1)
                    # wBot = wy1*M*(oy==0) + (1-wy1)*M*(oy==1)
                    ys = S(9); oy = S(5)  # wx0/o dead
                    nc.vector.tensor_scalar(ys[:], y0f[:], 0.0, float(FH - 2), op.max, op.min)
                    nc.vector.tensor_tensor(oy[:], y0f[:], ys[:], op.subtract)
                    wyA = S(6); wyB = S(4)  # wx1, y0f dead
                    nc.vector.tensor_scalar(t0[:], wy1[:], -1.0, 1.0, op.mult, op.add)
                    nc.vector.tensor_tensor(wyA[:], t0[:], M[:], op.mult)   # (1-wy1)*M
                    nc.vector.tensor_tensor(wyB[:], wy1[:], M[:], op.mult)  # wy1*M
                    wTop = S(8); wBot = S(3)  # wy1, x0f dead
                    nc.vector.tensor_single_scalar(e0[:], oy[:], 0.0, op.is_equal)
                    nc.vector.tensor_tensor(wTop[:], wyA[:], e0[:], op.mult)
                    nc.vector.tensor_tensor(wBot[:], wyB[:], e0[:], op.mult)
                    nc.vector.tensor_single_scalar(e0[:], oy[:], -1.0, op.is_equal)
                    nc.vector.tensor_tensor(t0[:], wyB[:], e0[:], op.mult)
                    nc.vector.tensor_tensor(wTop[:], wTop[:], t0[:], op.add)
                    nc.vector.tensor_single_scalar(e0[:], oy[:], 1.0, op.is_equal)
                    nc.vector.tensor_tensor(t0[:], wyA[:], e0[:], op.mult)
                    nc.vector.tensor_tensor(wBot[:], wBot[:], t0[:], op.add)

                    # 4 quad tap weights (order: TL, TR, BL, BR)
                    w00 = S(4); w10 = S(5); w01 = S(6); w11 = S(10)  # wyA/wyB/oy/e0 dead
                    nc.vector.tensor_tensor(w00[:], wL[:], wTop[:], op.mult)
                    nc.vector.tensor_tensor(w10[:], wR[:], wTop[:], op.mult)
                    nc.vector.tensor_tensor(w01[:], wL[:], wBot[:], op.mult)
                    nc.vector.tensor_tensor(w11[:], wR[:], wBot[:], op.mult)

                    # gather index: idx = camoff + ys*FW + xs  (exact ints)
                    idxA = S(3)  # wBot dead
                    nc.vector.scalar_tensor_tensor(idxA[:], ys[:], float(FW), xs[:], op.mult, op.add)
                    nc.vector.tensor_tensor(idxA[:], idxA[:], ck(12), op.add)
                    idxA16 = pl.tile([NSLOT, HALF], i16, tag="i0", name="i0")
                    nc.vector.tensor_copy(idxA16[:], idxA[:])
                    # idx write + wrapped readback + replicate on the SP
                    # queue (the wrap pattern needs the software DGE path);
                    # coefficient/grid loads moved to the ACT queue so the
                    # next half's loads aren't stuck behind this chain.
                    nc.sync.dma_start(
                        bass.AP(idx_dram.tensor, idx_dram.offset + h * NLIST * HALF,
                                [[HALF, NSLOT], [1, HALF]]), idxA16[:])
                    nc.sync.dma_start(
                        idxw[0:16, h, :],
                        bass.AP(idx_dram.tensor, idx_dram.offset + h * NLIST * HALF,
                                [[1, 16], [HALF, NLIST], [16, HALF // 16]]))
                    for g in range(1, 8):
                        nc.sync.dma_start(
                            idxw[16 * g:16 * (g + 1), h, :],
                            idxw[0:16, h, :])

                    # gathers for this half run under the other half's
                    # projection; accumulates are deferred past stage 1.
                    for s in range(NSLOT):
                        gA = pp.tile([128, NCH, 4 * C], fdt, tag="gA", name="gA",
                                     bufs=2)
                        gA_tiles.append(gA)
                        for c0, c1 in ((0, 8), (8, 11)):
                            ni = (c1 - c0) * 128
                            nc.gpsimd.dma_gather(
                                gA[:, c0:c1, :], featAP,
                                idxw[:, h, s * 88 + c0 * 8:s * 88 + c1 * 8],
                                ni, ni, 4 * C, elem_step=4 * C,
                                queue_num=(h * NSLOT + s) % 4)

                    # transpose 5 payloads per 128-q chunk into wT
                    for jj in range(NCH):
                        j = h * NCH + jj
                        ps = psp.tile([128, NW], f32, tag="tp", name="tp")
                        for k, w in enumerate((w00, w10, w01, w11, M)):
                            nc.tensor.transpose(
                                ps[:, k * NSLOT:(k + 1) * NSLOT],
                                w[:, jj * 128:(jj + 1) * 128], ident[0:NSLOT, 0:NSLOT])
                        nc.vector.tensor_copy(wT[:, j * NW:(j + 1) * NW], ps[:])

            # ---------------- stage 2: counts ------------------------------
            for j in range(NCHUNK):
                nc.vector.tensor_reduce(cnt[:, j:j + 1],
                                        wT[:, j * NW + 4 * NSLOT:j * NW + 5 * NSLOT],
                                        mybir.AxisListType.X, op.add)
            nc.vector.tensor_scalar_max(cnt[:], cnt[:], 1.0)
            nc.vector.reciprocal(rec[:], cnt[:])

            # ---------------- stage 3: accumulate ---------------------------
            for h in range(2):
                for s in range(NSLOT):
                    gA = gA_tiles[h * NSLOT + s]
                    for jj in range(NCH):
                        j = h * NCH + jj
                        w = lambda k: wT[:, j * NW + k * NSLOT + s:j * NW + k * NSLOT + s + 1]
                        for k in range(4):
                            nc.vector.scalar_tensor_tensor(
                                acc_nxt[:, j, :], gA[:, jj, k * C:(k + 1) * C],
                                w(k), acc_cur[:, j, :], op.mult, op.add)
                            acc_cur, acc_nxt = acc_nxt, acc_cur

            # ---------------- stage 4: normalize + write out ----------------
            for j in range(NCHUNK):
                nc.vector.tensor_scalar(accA[:, j, :], accA[:, j, :],
                                        rec[:, j:j + 1], 1.0, op.mult, op.mult)
            nc.sync.dma_start(
                bass.AP(outd, 0, [[C, 128], [128 * C, NCHUNK], [1, C]]), accA[:])

    nc.compile()
    return nc


def _get_program():
    if "nc" not in _CACHE:
        _CACHE["nc"] = _build_program()
    return _CACHE["nc"]


def _host_prep(feat, I, E, grid_3d):
    feat = np.asarray(feat, np.float32).reshape(NCAM, FH, FW, C)
    if not os.environ.get("BASS_KERNEL_F32"):
        import ml_dtypes
        feat = feat.astype(ml_dtypes.bfloat16)
    # 2x2 quad packing: featq[n, y, x, tap, c], taps (TL, TR, BL, BR);
    # edges clamp-duplicated (never read with nonzero weight).
    fx = np.concatenate([feat[:, :, 1:], feat[:, :, -1:]], axis=2)   # x+1
    fy = np.concatenate([feat[:, 1:], feat[:, -1:]], axis=1)         # y+1
    fxy = np.concatenate([fx[:, 1:], fx[:, -1:]], axis=1)            # x+1,y+1
    featq = np.stack([feat, fx, fy, fxy], axis=3).reshape(NPIX, 4 * C)
    featq = np.ascontiguousarray(featq)

    I = np.asarray(I, np.float64)[0]
    E = np.asarray(E, np.float64)[0]
    g = np.asarray(grid_3d, np.float32).reshape(DD, 3, QTOT)

    scale = (PC[3:6] - PC[0:3])
    off = PC[0:3]
    l2i = np.einsum('nij,njk->nik', I, E[:, :3, :])  # (6, 3, 4)
    # per-camera affine coefficients: 12 projection + camera pixel offset
    cc = np.zeros((NCAM, 13), np.float32)
    for n in range(NCAM):
        for r in range(3):
            cc[n, 4 * r:4 * r + 3] = (l2i[n, r, :3] * scale).astype(np.float32)
            cc[n, 4 * r + 3] = np.float32(l2i[n, r, :3] @ off + l2i[n, r, 3])
        cc[n, 12] = np.float32(n * PIX)

    gp = np.zeros((3, DD, QPAD), np.float32)
    gp[:, :, :QTOT] = g.transpose(1, 0, 2)

    # ---- per-query camera candidate selection (u/z validity is depth-
    # independent for horizontal-forward cameras; <=2 cams per query) ----
    X = gp[0, :, :]  # (DD, QPAD) normalized coords
    Y = gp[1, :, :]
    Z = gp[2, :, :]
    margin = 1e-3
    valid = np.zeros((NCAM, QPAD), bool)
    for n in range(NCAM):
        xc = cc[n, 0] * X + cc[n, 1] * Y + cc[n, 2] * Z + cc[n, 3]
        zc = cc[n, 8] * X + cc[n, 9] * Y + cc[n, 10] * Z + cc[n, 11]
        u = xc / np.maximum(zc, EPS) / IMG_W
        v_ok = (zc > EPS * 0.5) & (u > -margin) & (u < 1.0 + margin)
        valid[n] = v_ok.any(axis=0)
    nv = valid.sum(axis=0)
    assert nv.max() <= 2, f"camera selection overflow: {nv.max()} cams valid"
    cam = np.argsort(~valid, axis=0, kind='stable')[:2]  # (2, QPAD)

    # selected coefficients: [13, 2, QPAD]
    coefsel = np.ascontiguousarray(cc[cam].transpose(2, 0, 1))
    return featq, coefsel, gp


def kernel(feat, I, E, grid_3d):
    from concourse import bass_utils

    featq, coefsel, gp = _host_prep(feat, I, E, grid_3d)
    nc = _get_program()

    in_maps = []
    for c in range(NCORES):
        sl = slice(c * QCORE, (c + 1) * QCORE)
        in_maps.append({
            "feat": featq,
            "coef": np.ascontiguousarray(coefsel[:, :, sl]),
            "grid": np.ascontiguousarray(gp[:, :, sl]),
        })

    trace = bool(os.environ.get("BASS_KERNEL_TRACE"))
    if trace:
        import ntff_shim  # noqa: F401
    res = bass_utils.run_bass_kernel_spmd(nc, in_maps, core_ids=list(range(NCORES)),
                                          trace=trace)
    if trace:
        kernel.last_exec_time_ns = res.exec_time_ns

    out = np.concatenate([res.results[c]["out"] for c in range(NCORES)], axis=0)
    return out[:QTOT].reshape(1, QTOT, C)
